# revision 2
# baseline (speedup 1.0000x reference)
"""TRN2 Bass kernel for nn_DAGLayer (gnn_message_passing).

DAG of 1x1 convs over [B=64, C=64, H=32, W=32]:
  preproc: s0 = W_pre[0] @ x0, s1 = W_pre[1] @ x1   (channel matmul)
  node i (i=0..3): s_{2+i} = sum_j conv1x1(relu(s_j), W_edge[...]) over all
  prior states j; output = concat(s2..s5) on channels -> [B, 256, H, W].

Strategy: data-parallel over batch across 8 NeuronCores (8 batches/core).
Every 1x1 conv is a channel-dim matmul over the 1024 spatial columns of
one batch. One batch = one 1024-wide super-iteration: PSUM tiles span 2
banks ([128,1024] fp32) so every elementwise drain is a single wide
instruction (halves instruction + semaphore counts vs 512-wide tiling).

Engine split per batch:
  PE      12 matmuls (P, A1, B1, A2, B2, B3 x 2 column halves), B1 hoisted
          between A1 and A2 so the PE stays busy during the r2 relu; a
          dense queue keeps the PE at its 2.4 GHz pstate.
  scalar  relu01 (pP -> r01 fp16) + castA/castB (pA/pB -> fp16 out), all
          1024-wide PSUM passes at 1.2 GHz (Relu and Copy share one act
          table -> no table reloads).
  vector  r2/r4 relus straight from PSUM, r3 from the already-cast fp16
          outA in SBUF (hits the DVE 4-elem/cycle single-src mode).
Weights are packed host-side into a single [128, 640] fp16 lhsT block
(fp16 keeps the 10-bit mantissa; matmul rate is identical to bf16).
"""
import sys

sys.path.insert(0, '/opt/trn_rl_repo')

import numpy as np

N_CORES = 8
B, C, H, W_SP = 64, 64, 32, 32
BP = B // N_CORES          # batches per core
HW = H * W_SP              # 1024 spatial columns per batch

# Set by test harness to capture an NTFF trace; harmless default.
TRACE = False
LAST_RESULTS = None

_cache = {}


def _pack_weights(W_pre: np.ndarray, W_edge: np.ndarray) -> np.ndarray:
    """Pack all conv weights into one [128, 640] fp16 lhsT block.

    Layout (cols):
      0:128   WP  block-diag preproc: out [s0; s1] from rhs [x0; x1]
      128:256 A1  out [s2 | s3p] from rhs R01 = [r0; r1]
      256:384 B1  out [s4p | s5p] from rhs R01
      384:512 B2  out [s4p | s5p] from rhs R23 = [r2; r3]
      512:576 A2  (rows 0:64) edge r2->s3, written at PSUM partitions 64:128
      576:640 B3  (rows 0:64) edge r4->s5, written at PSUM partitions 64:128
    lhsT[k, m] = W[m, k] (pre-transposed for the PE's stationary operand).
    """
    Wt = np.zeros((128, 640), np.float32)
    T = lambda w: np.ascontiguousarray(w.T)
    Wt[0:64, 0:64] = T(W_pre[0])
    Wt[64:128, 64:128] = T(W_pre[1])
    # A1: cols 0:64 -> s2 (edges 0(r0), 1(r1)); cols 64:128 -> s3p (2, 3)
    Wt[0:64, 128:192] = T(W_edge[0])
    Wt[64:128, 128:192] = T(W_edge[1])
    Wt[0:64, 192:256] = T(W_edge[2])
    Wt[64:128, 192:256] = T(W_edge[3])
    # B1: cols 0:64 -> s4p (5(r0), 6(r1)); cols 64:128 -> s5p (9, 10)
    Wt[0:64, 256:320] = T(W_edge[5])
    Wt[64:128, 256:320] = T(W_edge[6])
    Wt[0:64, 320:384] = T(W_edge[9])
    Wt[64:128, 320:384] = T(W_edge[10])
    # B2 (rhs [r2; r3]): cols 0:64 -> s4p (7(r2), 8(r3)); cols 64:128 -> s5p (11, 12)
    Wt[0:64, 384:448] = T(W_edge[7])
    Wt[64:128, 384:448] = T(W_edge[8])
    Wt[0:64, 448:512] = T(W_edge[11])
    Wt[64:128, 448:512] = T(W_edge[12])
    # second-tier edges (K=64, weights at rows 0:64)
    Wt[0:64, 512:576] = T(W_edge[4])
    Wt[0:64, 576:640] = T(W_edge[13])
    return Wt.astype(np.float16)


def _build_program():
    import concourse.tile as tile
    from concourse import bacc, mybir

    F16, F32 = mybir.dt.float16, mybir.dt.float32
    Relu = mybir.ActivationFunctionType.Relu
    Copy = mybir.ActivationFunctionType.Copy

    nc = bacc.Bacc()
    X = nc.dram_tensor("X", [BP, 128, HW], F16, kind="ExternalInput")
    Wt = nc.dram_tensor("Wt", [128, 640], F16, kind="ExternalInput")
    O = nc.dram_tensor("O", [BP, 256, HW], F16, kind="ExternalOutput")

    HALVES = (slice(0, 512), slice(512, 1024))
    with tile.TileContext(nc) as tc:
        with tc.tile_pool(name="wpool", bufs=1) as wpool, \
             tc.tile_pool(name="xpool", bufs=3) as xpool, \
             tc.tile_pool(name="rpool", bufs=2) as rpool, \
             tc.tile_pool(name="opool", bufs=2) as opool, \
             tc.tile_pool(name="ppool", bufs=1, space="PSUM") as ppool, \
             tc.tile_pool(name="apool", bufs=1, space="PSUM") as apool, \
             tc.tile_pool(name="bpool", bufs=2, space="PSUM") as bpool:
            w = wpool.tile([128, 640], F16, tag="w")
            nc.gpsimd.dma_start(w[:], Wt[:])

            xs = [None] * BP

            def load(b):
                if b < BP:
                    xs[b] = xpool.tile([128, HW], F16, tag="x", name="x")
                    nc.gpsimd.dma_start(xs[b][:], X[b])

            load(0)
            load(1)
            for b in range(BP):
                load(b + 2)
                x = xs[b]
                # preproc: pP = [s0; s1], block-diag K=128 matmul
                pP = ppool.tile([128, HW], F32, tag="pP")
                for s in HALVES:
                    nc.tensor.matmul(pP[:, s], w[:, 0:128], x[:, s],
                                     start=True, stop=True)
                r01 = rpool.tile([128, HW], F16, tag="r01")
                nc.scalar.activation(r01[:], pP[:], Relu)

                pA = apool.tile([128, HW], F32, tag="pA")
                pB = bpool.tile([128, HW], F32, tag="pB")
                # A1: pA = [s2; s3p] from r01
                for s in HALVES:
                    nc.tensor.matmul(pA[:, s], w[:, 128:256], r01[:, s],
                                     start=True, stop=False)
                # B1: pB = [s4p; s5p] from r01 (keeps PE busy during r2)
                for s in HALVES:
                    nc.tensor.matmul(pB[:, s], w[:, 256:384], r01[:, s],
                                     start=True, stop=False)
                r23 = rpool.tile([128, HW], F16, tag="r23")
                nc.vector.tensor_relu(r23[0:64, :], pA[0:64, :])       # r2
                # A2: s3 += e4 @ r2 into PSUM partitions 64:128
                for s in HALVES:
                    nc.tensor.matmul(pA[64:128, s], w[0:64, 512:576],
                                     r23[0:64, s], start=False, stop=True,
                                     tile_position=(0, 64))
                outA = opool.tile([128, HW], F16, tag="outA")
                nc.scalar.activation(outA[:], pA[:], Copy)
                # r3 from the fp16 SBUF copy (DVE 4-elem/cycle mode)
                nc.vector.tensor_relu(r23[64:128, :], outA[64:128, :])  # r3
                nc.sync.dma_start(O[b, 0:128, :], outA[:])
                # B2: pB += [s4p; s5p] from [r2; r3]
                for s in HALVES:
                    nc.tensor.matmul(pB[:, s], w[:, 384:512], r23[:, s],
                                     start=False, stop=False)
                r4 = rpool.tile([64, HW], F16, tag="r4")
                nc.vector.tensor_relu(r4[:], pB[0:64, :])               # r4
                # B3: s5 += e13 @ r4 into PSUM partitions 64:128
                for s in HALVES:
                    nc.tensor.matmul(pB[64:128, s], w[0:64, 576:640],
                                     r4[0:64, s], start=False, stop=True,
                                     tile_position=(0, 64))
                outB = opool.tile([128, HW], F16, tag="outB")
                nc.scalar.activation(outB[:], pB[:], Copy)
                nc.sync.dma_start(O[b, 128:256, :], outB[:])
    nc.compile()
    return nc


def _get_program():
    if "nc" not in _cache:
        _cache["nc"] = _build_program()
    return _cache["nc"]


def kernel(x0, x1, W_pre, W_edge):
    global LAST_RESULTS
    from concourse.bass_utils import run_bass_kernel_spmd

    nc = _get_program()
    Xp = np.concatenate(
        [x0.reshape(B, C, HW), x1.reshape(B, C, HW)], axis=1)   # [B, 128, HW]
    Xp = Xp.astype(np.float16)
    Wt = _pack_weights(np.asarray(W_pre, np.float32), np.asarray(W_edge, np.float32))
    in_maps = [
        {"X": np.ascontiguousarray(Xp[i * BP:(i + 1) * BP]), "Wt": Wt}
        for i in range(N_CORES)
    ]
    res = run_bass_kernel_spmd(nc, in_maps, core_ids=list(range(N_CORES)),
                               trace=TRACE)
    LAST_RESULTS = res
    out = np.concatenate([r["O"] for r in res.results], axis=0).astype(np.float32)
    return np.ascontiguousarray(out.reshape(B, 4 * C, H, W_SP))
